# revision 8
# baseline (speedup 1.0000x reference)
"""Multi-head latent attention (MLA) Trainium2 kernel, 8-core SPMD.

Strategy:
  - tokens flat T=4096 (B=2 x S=2048); core c owns token shard [512c, 512c+512)
  - phase A (token-parallel): latent down-proj latT = w_cat^T @ xT on own shard
    (w_cat = [w_qkv | w_kpos | w_kpos_rot]); RoPE pos_k on shard; AllGather.
  - phase B (head-parallel, heads {2c, 2c+1}): up-projections from gathered
    latents, producing feature-major qT/kT [128, 4096] per head, token-major v,
    and RoPE'd positional queries (rotation done via pre-permuted weight copy:
    rope(u) = u*cos + perm(u)*sin_signed, perm baked into w_qpos_rot).
  - phase C: attention in transposed orientation scoresT[k, q] so that
    pT = exp(scoresT*scale) feeds attnT = v^T @ pT directly; denominators via
    ones-column matmul; softmax is max-free (scores are bounded, fp32 exp).
  - AllToAll redistributes attnT from head-major to token-major shards.
  - phase D (token-parallel): o_proj + bias on own token shard; host concat.
All matmul operands bf16, fp32 PSUM accumulation.
"""
import numpy as np
import ml_dtypes

import concourse.bass as bass
import concourse.bacc as bacc
import concourse.mybir as mybir
import concourse.tile as tile
from concourse.bass_utils import run_bass_kernel_spmd

F32 = mybir.dt.float32
BF16 = mybir.dt.bfloat16
AF = mybir.ActivationFunctionType
OP = mybir.AluOpType
BF = ml_dtypes.bfloat16

MODEL = 2048
LATENT = 512
POS = 1024
NH = 16
HD = 128          # head dim (main)
PHD = 64          # positional head dim
THETA = 50000.0
B = 2
S = 2048
T = B * S         # 4096 flat tokens
NC = 8            # cores
TS = T // NC      # 512 tokens per core
HC = NH // NC     # 2 heads per core
SCALE = 1.0 / float(np.sqrt(HD + PHD))

LQ_TILES = LATENT // 128          # 4 l-chunks per latent
N_LAT_TILES = 3 * LQ_TILES + 1    # 12 latent tiles + 1 posk-pack tile
AG_ROWS = 3 * LATENT + PHD        # 1600 rows per rank in latent AllGather

_ROT = np.r_[32:64, 0:32]         # within-64 rotate-half permutation

_CACHE = {}


def _build():
    nc = bacc.Bacc("TRN2", target_bir_lowering=False, debug=False,
                   num_devices=NC)

    # ---- I/O ----
    xT = nc.dram_tensor("xT", [MODEL, TS], BF16, kind="ExternalInput")
    w_cat = nc.dram_tensor("w_cat", [MODEL, 3 * LATENT + 2 * PHD], BF16,
                           kind="ExternalInput")
    b_cat = nc.dram_tensor("b_cat", [128, N_LAT_TILES], F32,
                           kind="ExternalInput")
    w_qup = nc.dram_tensor("w_qup", [LATENT, HC * HD], BF16,
                           kind="ExternalInput")
    w_kup = nc.dram_tensor("w_kup", [LATENT, HC * HD], BF16,
                           kind="ExternalInput")
    w_vup = nc.dram_tensor("w_vup", [LATENT, HC * HD], BF16,
                           kind="ExternalInput")
    w_qpos = nc.dram_tensor("w_qpos", [LATENT, HC * PHD], BF16,
                            kind="ExternalInput")
    w_qposr = nc.dram_tensor("w_qposr", [LATENT, HC * PHD], BF16,
                             kind="ExternalInput")
    b_qk = nc.dram_tensor("b_qk", [128, 4], F32, kind="ExternalInput")
    b_qpos = nc.dram_tensor("b_qpos", [128, 2], F32, kind="ExternalInput")
    b_vrow = nc.dram_tensor("b_vrow", [1, HC * HD], BF16, kind="ExternalInput")
    b_orow = nc.dram_tensor("b_orow", [1, MODEL], BF16, kind="ExternalInput")
    cos2 = nc.dram_tensor("cos2", [128, T], BF16, kind="ExternalInput")
    sin2 = nc.dram_tensor("sin2", [128, T], BF16, kind="ExternalInput")
    sc_sh = nc.dram_tensor("sc_sh", [128, TS], F32, kind="ExternalInput")
    tri = nc.dram_tensor("tri", [128, 128], BF16, kind="ExternalInput")
    w_o = nc.dram_tensor("w_o", [MODEL, MODEL], BF16, kind="ExternalInput")
    out_sh = nc.dram_tensor("out_sh", [TS, MODEL], F32, kind="ExternalOutput")

    with tile.TileContext(nc) as tc:
        with (
            tc.tile_pool(name="const", bufs=1) as cpool,
            tc.tile_pool(name="resq", bufs=1) as rpool,
            tc.tile_pool(name="work", bufs=3) as wpool,
            tc.tile_pool(name="psum", bufs=1, space="PSUM") as pspool,
            tc.tile_pool(name="dram", bufs=1, space="DRAM") as dram,
        ):
            # ---------- constants ----------
            b_cat_sb = cpool.tile([128, N_LAT_TILES], F32, tag="bcat")
            nc.sync.dma_start(out=b_cat_sb[:], in_=b_cat.ap())
            b_qk_sb = cpool.tile([128, 4], F32, tag="bqk")
            nc.sync.dma_start(out=b_qk_sb[:], in_=b_qk.ap())
            b_qpos_sb = cpool.tile([128, 2], F32, tag="bqpos")
            nc.sync.dma_start(out=b_qpos_sb[:], in_=b_qpos.ap())
            b_vrow_sb = cpool.tile([1, HC * HD], BF16, tag="bvrow")
            nc.sync.dma_start(out=b_vrow_sb[:], in_=b_vrow.ap())
            b_orow_sb = cpool.tile([1, MODEL], BF16, tag="borow")
            nc.sync.dma_start(out=b_orow_sb[:], in_=b_orow.ap())
            cos2_sb = cpool.tile([128, T], BF16, tag="cos2")
            nc.sync.dma_start(out=cos2_sb[:], in_=cos2.ap())
            sin2_sb = cpool.tile([128, T], BF16, tag="sin2")
            nc.sync.dma_start(out=sin2_sb[:], in_=sin2.ap())
            sc_sh_sb = cpool.tile([128, TS], F32, tag="scsh")
            nc.sync.dma_start(out=sc_sh_sb[:], in_=sc_sh.ap())
            tri_sb = cpool.tile([128, 128], BF16, tag="tri")
            nc.sync.dma_start(out=tri_sb[:], in_=tri.ap())
            ones_row = cpool.tile([1, 128], BF16, tag="onesr")
            nc.vector.memset(ones_row[:], 1.0)
            ones_col = cpool.tile([128, 1], BF16, tag="onesc")
            nc.vector.memset(ones_col[:], 1.0)

            # up-proj weights resident (small)
            w_qup_sb, w_kup_sb, w_vup_sb, w_qpos_sb, w_qposr_sb = [], [], [], [], []
            for j in range(LQ_TILES):
                for lst, src, width, nm in (
                    (w_qup_sb, w_qup, HC * HD, "wqup"),
                    (w_kup_sb, w_kup, HC * HD, "wkup"),
                    (w_vup_sb, w_vup, HC * HD, "wvup"),
                    (w_qpos_sb, w_qpos, HC * PHD, "wqpos"),
                    (w_qposr_sb, w_qposr, HC * PHD, "wqposr"),
                ):
                    t_ = cpool.tile([128, width], BF16, tag=f"{nm}{j}")
                    nc.sync.dma_start(
                        out=t_[:], in_=src.ap()[128 * j:128 * j + 128, :])
                    lst.append(t_)

            # collective bounce buffers
            ag_in = dram.tile([AG_ROWS, TS], BF16)
            ag_out = dram.tile([NC * AG_ROWS, TS], BF16, addr_space="Shared")
            a2a_in = dram.tile([NH * HD, TS], BF16)
            a2a_out = dram.tile([NH * HD, TS], BF16)

            # ---------- phase A: down-projection on own token shard ----------
            xT_sb = []
            for m in range(16):
                t_ = rpool.tile([128, TS], BF16, tag=f"xT{m}")
                nc.sync.dma_start(out=t_[:],
                                  in_=xT.ap()[128 * m:128 * m + 128, :])
                xT_sb.append(t_)

            for j in range(N_LAT_TILES):
                ps = pspool.tile([128, TS], F32, tag="ps512", bufs=6)
                for m in range(16):
                    wt = wpool.tile([128, 128], BF16, tag="wA", bufs=4)
                    nc.sync.dma_start(
                        out=wt[:],
                        in_=w_cat.ap()[128 * m:128 * m + 128,
                                       128 * j:128 * j + 128])
                    nc.tensor.matmul(ps[:], wt[:], xT_sb[m][:],
                                     start=(m == 0), stop=(m == 15))
                if j < 3 * LQ_TILES:
                    lat = wpool.tile([128, TS], BF16, tag="latA", bufs=3)
                    nc.vector.tensor_scalar_add(lat[:], ps[:],
                                                b_cat_sb[:, j:j + 1])
                    nc.sync.dma_start(out=ag_in[128 * j:128 * j + 128, :],
                                      in_=lat[:])
                else:
                    # posk pack tile: rows 0:64 raw, 64:128 pre-rotated; RoPE
                    # t3 = (raw + b) * cos; t4 = (rot + b_rot) * sin_signed
                    # (PSUM in0 exempts the equal-base-partition SBUF rule;
                    #  t4's output lands at base 0.)
                    t3 = wpool.tile([PHD, TS], F32, tag="pk3", bufs=1)
                    t4 = wpool.tile([PHD, TS], F32, tag="pk4", bufs=1)
                    nc.vector.scalar_tensor_tensor(
                        t3[:], ps[0:PHD, :], b_cat_sb[0:PHD, j:j + 1],
                        sc_sh_sb[0:PHD, :], OP.add, OP.mult)
                    nc.vector.scalar_tensor_tensor(
                        t4[:], ps[PHD:128, :], b_cat_sb[PHD:128, j:j + 1],
                        sc_sh_sb[PHD:128, :], OP.add, OP.mult)
                    pk = wpool.tile([PHD, TS], BF16, tag="pk", bufs=1)
                    nc.vector.tensor_tensor(pk[:], t3[:], t4[:], OP.add)
                    nc.sync.dma_start(
                        out=ag_in[3 * LATENT:3 * LATENT + PHD, :], in_=pk[:])

            nc.gpsimd.collective_compute(
                "AllGather", OP.bypass,
                ins=[ag_in.opt()], outs=[ag_out.opt()],
                replica_groups=[list(range(NC))])

            # ---------- phase B: up-projections (local heads) ----------
            qT_sb = [rpool.tile([128, T], BF16, tag=f"qT{i}", name=f"qT{i}")
                     for i in range(HC)]
            kT_sb = [rpool.tile([128, T], BF16, tag=f"kT{i}", name=f"kT{i}")
                     for i in range(HC)]
            qpos_sb = rpool.tile([128, T], BF16, tag="qposT")  # 2 heads packed
            posk2_sb = rpool.tile([128, T], BF16, tag="posk2")  # duplicated
            v_sb = [[rpool.tile([128, HD], BF16, tag=f"v{i}_{tt}", name=f"v{i}_{tt}")
                     for tt in range(T // 128)] for i in range(HC)]

            for r in range(NC):
                cols = slice(TS * r, TS * r + TS)
                base = AG_ROWS * r
                # -- lq chunks
                lq = []
                for j in range(LQ_TILES):
                    t_ = wpool.tile([128, TS], BF16, tag="latB", bufs=5)
                    nc.sync.dma_start(
                        out=t_[:],
                        in_=ag_out[base + 128 * j:base + 128 * j + 128, :])
                    lq.append(t_)
                # q main, both heads
                for i in range(HC):
                    ps = pspool.tile([128, TS], F32, tag="ps512", bufs=6)
                    for j in range(LQ_TILES):
                        nc.tensor.matmul(
                            ps[:], w_qup_sb[j][:, HD * i:HD * i + HD],
                            lq[j][:], start=(j == 0), stop=(j == LQ_TILES - 1))
                    nc.vector.tensor_scalar_add(qT_sb[i][:, cols], ps[:],
                                                b_qk_sb[:, i:i + 1])
                # q pos (raw & rot), heads packed; rope combine
                ps_p = pspool.tile([128, TS], F32, tag="ps512", bufs=6)
                ps_pr = pspool.tile([128, TS], F32, tag="ps512", bufs=6)
                for j in range(LQ_TILES):
                    nc.tensor.matmul(ps_p[:], w_qpos_sb[j][:], lq[j][:],
                                     start=(j == 0), stop=(j == LQ_TILES - 1))
                for j in range(LQ_TILES):
                    nc.tensor.matmul(ps_pr[:], w_qposr_sb[j][:], lq[j][:],
                                     start=(j == 0), stop=(j == LQ_TILES - 1))
                t3 = wpool.tile([128, TS], F32, tag="qp3", bufs=2)
                t4 = wpool.tile([128, TS], F32, tag="qp4", bufs=2)
                nc.vector.scalar_tensor_tensor(
                    t3[:], ps_p[:], b_qpos_sb[:, 0:1], cos2_sb[:, cols],
                    OP.add, OP.mult)
                nc.vector.scalar_tensor_tensor(
                    t4[:], ps_pr[:], b_qpos_sb[:, 1:2], sin2_sb[:, cols],
                    OP.add, OP.mult)
                nc.vector.tensor_tensor(qpos_sb[:, cols], t3[:], t4[:], OP.add)
                # -- lk chunks -> kT
                lk = []
                for j in range(LQ_TILES):
                    t_ = wpool.tile([128, TS], BF16, tag="latB", bufs=5)
                    nc.sync.dma_start(
                        out=t_[:],
                        in_=ag_out[base + LATENT + 128 * j:
                                   base + LATENT + 128 * j + 128, :])
                    lk.append(t_)
                for i in range(HC):
                    ps = pspool.tile([128, TS], F32, tag="ps512", bufs=6)
                    for j in range(LQ_TILES):
                        nc.tensor.matmul(
                            ps[:], w_kup_sb[j][:, HD * i:HD * i + HD],
                            lk[j][:], start=(j == 0), stop=(j == LQ_TILES - 1))
                    nc.vector.tensor_scalar_add(kT_sb[i][:, cols], ps[:],
                                                b_qk_sb[:, 2 + i:3 + i])
                # -- lv chunks -> v (token-major)
                lv = []
                for j in range(LQ_TILES):
                    t_ = wpool.tile([128, TS], BF16, tag="latB", bufs=5)
                    nc.sync.dma_start(
                        out=t_[:],
                        in_=ag_out[base + 2 * LATENT + 128 * j:
                                   base + 2 * LATENT + 128 * j + 128, :])
                    lv.append(t_)
                for tt in range(TS // 128):
                    for i in range(HC):
                        psv = pspool.tile([128, HD], F32, tag="ps128", bufs=2)
                        nc.tensor.matmul(psv[:], ones_row[:],
                                         b_vrow_sb[:, HD * i:HD * i + HD],
                                         start=True, stop=False)
                        for j in range(LQ_TILES):
                            nc.tensor.matmul(
                                psv[:], lv[j][:, 128 * tt:128 * tt + 128],
                                w_vup_sb[j][:, HD * i:HD * i + HD],
                                start=False, stop=(j == LQ_TILES - 1))
                        TT = (TS // 128) * r + tt
                        nc.scalar.copy(v_sb[i][TT][:], psv[:])
                # -- pos_k rows -> both halves of posk2
                nc.sync.dma_start(
                    out=posk2_sb[0:PHD, cols],
                    in_=ag_out[base + 3 * LATENT:base + 3 * LATENT + PHD, :])
                nc.sync.dma_start(
                    out=posk2_sb[PHD:128, cols],
                    in_=ag_out[base + 3 * LATENT:base + 3 * LATENT + PHD, :])

            # ---------- phase C: attention ----------
            for g in range(B):
                for u in range(S // TS):
                    for i in range(HC):
                        qc0 = S * g + TS * u
                        tmax = 4 * u + 3
                        ps_at = pspool.tile([128, TS], F32, tag="ps512", bufs=6)
                        ps_sum = pspool.tile([1, TS], F32, tag="ps512", bufs=6)
                        for t in range(tmax + 1):
                            off = 128 * t - TS * u
                            qlo = max(0, off)
                            kc = S * g + 128 * t
                            qs = slice(qlo, TS)
                            ps_sc = pspool.tile([128, TS], F32, tag="ps512",
                                                bufs=6)
                            nc.tensor.matmul(
                                ps_sc[:, qs], kT_sb[i][:, kc:kc + 128],
                                qT_sb[i][:, qc0 + qlo:qc0 + TS],
                                start=True, stop=False)
                            nc.tensor.matmul(
                                ps_sc[:, qs],
                                posk2_sb[PHD * i:PHD * i + PHD, kc:kc + 128],
                                qpos_sb[PHD * i:PHD * i + PHD,
                                        qc0 + qlo:qc0 + TS],
                                start=False, stop=True)
                            pt = wpool.tile([128, TS], BF16, tag="pt", bufs=4)
                            nc.scalar.activation(pt[:, qs], ps_sc[:, qs],
                                                 AF.Exp, scale=SCALE)
                            if off >= 0:
                                nc.vector.tensor_tensor(
                                    pt[:, qlo:qlo + 128],
                                    pt[:, qlo:qlo + 128], tri_sb[:], OP.mult)
                            TT = (S // 128) * g + t
                            nc.tensor.matmul(ps_at[:, qs], v_sb[i][TT][:],
                                             pt[:, qs], start=(t == 0),
                                             stop=(t == tmax))
                            nc.tensor.matmul(ps_sum[:, qs], ones_col[:],
                                             pt[:, qs], start=(t == 0),
                                             stop=(t == tmax))
                        recf = wpool.tile([1, TS], F32, tag="recf", bufs=2)
                        nc.vector.reciprocal(recf[:], ps_sum[0:1, :])
                        recb = wpool.tile([1, TS], BF16, tag="recb", bufs=2)
                        nc.scalar.copy(recb[:], recf[:])
                        ps_rb = pspool.tile([128, TS], F32, tag="ps512",
                                            bufs=6)
                        nc.tensor.matmul(ps_rb[:], ones_row[:], recb[:],
                                         start=True, stop=True)
                        atmp = wpool.tile([128, TS], BF16, tag="atmp", bufs=2)
                        nc.scalar.copy(atmp[:], ps_at[:])
                        aout = wpool.tile([128, TS], BF16, tag="aout", bufs=2)
                        nc.vector.tensor_tensor(aout[:], atmp[:], ps_rb[:],
                                                OP.mult)
                        blk = (S // TS) * g + u
                        nc.sync.dma_start(
                            out=a2a_in[HC * HD * blk + HD * i:
                                       HC * HD * blk + HD * i + HD, :],
                            in_=aout[:])

            nc.gpsimd.collective_compute(
                "AllToAll", OP.bypass,
                ins=[a2a_in.opt()], outs=[a2a_out.opt()],
                replica_groups=[list(range(NC))])

            # ---------- phase D: o-projection on own token shard ----------
            ach = []
            for f in range(NH):
                row = HC * HD * (f // HC) + HD * (f % HC)
                per_tt = []
                for tt in range(TS // 128):
                    t_ = rpool.tile([128, 128], BF16, tag=f"ach{f}_{tt}")
                    nc.sync.dma_start(
                        out=t_[:],
                        in_=a2a_out[row:row + HD, 128 * tt:128 * tt + 128])
                    per_tt.append(t_)
                ach.append(per_tt)

            for oc in range(MODEL // TS):
                ocs = slice(TS * oc, TS * oc + TS)
                psd = []
                for tt in range(TS // 128):
                    ps = pspool.tile([128, TS], F32, tag="ps512", bufs=6)
                    nc.tensor.matmul(ps[:], ones_row[:], b_orow_sb[:, ocs],
                                     start=True, stop=False)
                    psd.append(ps)
                for f in range(NH):
                    wt = wpool.tile([128, TS], BF16, tag="wD", bufs=3)
                    nc.sync.dma_start(
                        out=wt[:], in_=w_o.ap()[128 * f:128 * f + 128, ocs])
                    for tt in range(TS // 128):
                        nc.tensor.matmul(psd[tt][:], ach[f][tt][:], wt[:],
                                         start=False, stop=(f == NH - 1))
                for tt in range(TS // 128):
                    osb = wpool.tile([128, TS], F32, tag="osb", bufs=2)
                    nc.vector.tensor_copy(osb[:], psd[tt][:])
                    nc.sync.dma_start(
                        out=out_sh.ap()[128 * tt:128 * tt + 128, ocs],
                        in_=osb[:])

    nc.compile()
    return nc


def _host_prep(inputs):
    x = inputs["x"]
    w_qkv, b_qkv = inputs["w_qkv"], inputs["b_qkv"]
    w_qup, b_qup = inputs["w_qup"], inputs["b_qup"]
    w_kup, b_kup = inputs["w_kup"], inputs["b_kup"]
    w_vup, b_vup = inputs["w_vup"], inputs["b_vup"]
    w_qpos, b_qpos = inputs["w_qpos"], inputs["b_qpos"]
    w_kpos, b_kpos = inputs["w_kpos"], inputs["b_kpos"]
    w_o, b_o = inputs["w_o"], inputs["b_o"]

    x_flat = np.asarray(x, np.float32).reshape(T, MODEL)

    # rope tables
    inv_freq = 1.0 / (THETA ** (np.arange(0, PHD, 2, dtype=np.float32) / PHD))
    pos = np.arange(S, dtype=np.float32)
    freqs = np.outer(pos, inv_freq)                    # [S, 32]
    emb = np.concatenate([freqs, freqs], -1)           # [S, 64]
    cos = np.cos(emb).astype(np.float32)               # [S, 64]
    sin = np.sin(emb).astype(np.float32)
    sin_signed = np.concatenate([-sin[:, :32], sin[:, 32:]], -1)
    cosT = np.concatenate([cos, cos], 0).T             # [64, T]
    sinT = np.concatenate([sin_signed, sin_signed], 0).T
    cos2 = np.concatenate([cosT, cosT], 0).astype(BF)  # [128, T]
    sin2 = np.concatenate([sinT, sinT], 0).astype(BF)

    # w_cat = [w_qkv | w_kpos | w_kpos_rot]
    w_kpos_rot = w_kpos[:, _ROT]
    w_cat = np.concatenate([w_qkv, w_kpos, w_kpos_rot], 1).astype(BF)

    # b_cat per-partition bias: 12 latent cols + posk pack col
    b_cat = np.zeros((128, N_LAT_TILES), np.float32)
    for j in range(3 * LQ_TILES):
        b_cat[:, j] = b_qkv[128 * j:128 * j + 128]
    b_cat[0:PHD, 12] = b_kpos
    b_cat[PHD:128, 12] = b_kpos[_ROT]

    tri = np.triu(np.ones((128, 128), np.float32)).astype(BF)  # [kk<=qq]

    common = {
        "w_cat": w_cat, "b_cat": b_cat,
        "cos2": cos2, "sin2": sin2, "tri": tri,
        "w_o": np.asarray(w_o, np.float32).astype(BF),
        "b_orow": np.asarray(b_o, np.float32).reshape(1, MODEL).astype(BF),
    }

    in_maps = []
    for c in range(NC):
        h0 = HC * c
        cols_m = slice(HD * h0, HD * h0 + HC * HD)
        cols_p = slice(PHD * h0, PHD * h0 + HC * PHD)
        wqp = w_qpos[:, cols_p]
        wqpr = np.concatenate(
            [wqp[:, PHD * i:PHD * i + PHD][:, _ROT] for i in range(HC)], 1)
        b_qk_t = np.zeros((128, 4), np.float32)
        b_qpos_t = np.zeros((128, 2), np.float32)
        for i in range(HC):
            b_qk_t[:, i] = b_qup[HD * (h0 + i):HD * (h0 + i) + HD]
            b_qk_t[:, 2 + i] = b_kup[HD * (h0 + i):HD * (h0 + i) + HD]
            bq = b_qpos[PHD * (h0 + i):PHD * (h0 + i) + PHD]
            b_qpos_t[PHD * i:PHD * i + PHD, 0] = bq
            b_qpos_t[PHD * i:PHD * i + PHD, 1] = bq[_ROT]
        tok = slice(TS * c, TS * c + TS)
        m = {
            "xT": np.ascontiguousarray(x_flat[tok].T).astype(BF),
            "w_qup": np.asarray(w_qup[:, cols_m], np.float32).astype(BF),
            "w_kup": np.asarray(w_kup[:, cols_m], np.float32).astype(BF),
            "w_vup": np.asarray(w_vup[:, cols_m], np.float32).astype(BF),
            "w_qpos": np.asarray(wqp, np.float32).astype(BF),
            "w_qposr": np.asarray(wqpr, np.float32).astype(BF),
            "b_qk": b_qk_t, "b_qpos": b_qpos_t,
            "b_vrow": np.asarray(b_vup[cols_m], np.float32)
                        .reshape(1, -1).astype(BF),
            "sc_sh": np.ascontiguousarray(
                np.concatenate([cosT[:, tok], sinT[:, tok]], 0)
            ).astype(np.float32),
        }
        m.update(common)
        in_maps.append(m)
    return in_maps


def kernel(**inputs) -> np.ndarray:
    if "nc" not in _CACHE:
        _CACHE["nc"] = _build()
    nc = _CACHE["nc"]
    in_maps = _host_prep({k: np.asarray(v) for k, v in inputs.items()})
    res = run_bass_kernel_spmd(nc, in_maps, list(range(NC))).results
    out = np.concatenate([res[c]["out_sh"] for c in range(NC)], 0)
    return out.reshape(B, S, MODEL).astype(np.float32)


# revision 13
# speedup vs baseline: 1.3041x; 1.3041x over previous
"""Multi-head latent attention (MLA) Trainium2 kernel, 8-core SPMD.

Sharding: cores split into 2 batch-groups of 4 (cores 0-3 = batch 0,
4-7 = batch 1). Within a group, core w owns token shard [512w, 512w+512)
of its batch and heads {4w..4w+3}.

  - phase A (token-parallel): latent down-proj latT = w_cat^T @ xT on own
    shard (w_cat = [w_qkv | w_kpos | w_kpos_rot]); RoPE pos_k; group
    AllGather in partition-major layout (one DMA per rank downstream).
  - phase B (head-parallel): up-projections from gathered latents:
    feature-major qT/kT [128, 2048] per head, token-major v, RoPE'd
    positional queries (rotation via pre-permuted weight copy:
    rope(u) = u*cos + perm(u)*sin_signed).
  - phase C: attention in transposed orientation scoresT[k, q]:
    pT = exp(scoresT*scale) feeds attnT = v^T @ pT directly; denominators
    via ones-column matmul; max-free softmax (scores bounded, fp32 exp).
    attnT stays resident in SBUF.
  - phase D: partial o_proj over local heads for ALL batch tokens
    (+ b_o/4 so the group sum restores the bias once), then 2x
    ReduceScatter (split by output-column halves for overlap) hands each
    core its own token shard, summed over the group's heads.
All matmul operands bf16, fp32 PSUM accumulation. Host assembles shards.
"""
import numpy as np
import ml_dtypes

import concourse.bacc as bacc
import concourse.mybir as mybir
import concourse.tile as tile
from concourse.bass_utils import run_bass_kernel_spmd

F32 = mybir.dt.float32
BF16 = mybir.dt.bfloat16
AF = mybir.ActivationFunctionType
OP = mybir.AluOpType
BF = ml_dtypes.bfloat16

MODEL = 2048
LATENT = 512
NH = 16
HD = 128          # head dim (main)
PHD = 64          # positional head dim
THETA = 50000.0
B = 2
S = 2048
T = B * S
NC = 8
G = 4             # cores per batch-group
TS = T // NC      # 512 tokens per core shard
HC = NH // G      # 4 heads per core
SCALE = 1.0 / float(np.sqrt(HD + PHD))

LJ = LATENT // 128                # 4 l-chunks per latent
NLT = 3 * LJ + 1                  # 12 latent tiles + 1 posk-pack tile
AGW = NLT * TS                    # 6656 ag width (partition-major)
NU = S // TS                      # 4 q spans per batch

# bias views into bcon: cols [0:13] b_cat, then q heads, k heads, qpos packs
BQ0, BK0, BP0 = NLT, NLT + HC, NLT + 2 * HC
# wup col layout per j-chunk (stride 2048)
WQ, WK, WV, WP, WPR = 0, 512, 1024, 1536, 1792

_ROT = np.r_[32:64, 0:32]

_CACHE = {}


def _build():
    nc = bacc.Bacc("TRN2", target_bir_lowering=False, debug=False,
                   num_devices=NC)

    xT = nc.dram_tensor("xT", [128, 16 * TS], BF16, kind="ExternalInput")
    w_catp = nc.dram_tensor("w_catp", [128, NLT * 2048], BF16,
                            kind="ExternalInput")
    wup = nc.dram_tensor("wup", [128, LJ * 2048], BF16, kind="ExternalInput")
    wol = nc.dram_tensor("wol", [128, HC * MODEL], BF16, kind="ExternalInput")
    bcon = nc.dram_tensor("bcon", [128, BP0 + 4], F32, kind="ExternalInput")
    bvb = nc.dram_tensor("bvb", [128, HC * HD], BF16, kind="ExternalInput")
    bob = nc.dram_tensor("bob", [128, MODEL], BF16, kind="ExternalInput")
    sc2 = nc.dram_tensor("sc2", [128, 2 * S], BF16, kind="ExternalInput")
    sc_sh = nc.dram_tensor("sc_sh", [128, TS], F32, kind="ExternalInput")
    tri = nc.dram_tensor("tri", [128, 128], BF16, kind="ExternalInput")
    out_sh = nc.dram_tensor("out_sh", [TS, MODEL], F32, kind="ExternalOutput")

    groups = [[0, 1, 2, 3], [4, 5, 6, 7]]

    with tile.TileContext(nc) as tc:
        with (
            tc.tile_pool(name="const", bufs=1) as cpool,
            tc.tile_pool(name="psum", bufs=1, space="PSUM") as pspool,
            tc.tile_pool(name="dram", bufs=1, space="DRAM") as dram,
        ):
            # ---------- constants ----------
            bcon_sb = cpool.tile([128, BP0 + 4], F32, tag="bcon")
            nc.sync.dma_start(out=bcon_sb[:], in_=bcon.ap())
            bvb_sb = cpool.tile([128, HC * HD], BF16, tag="bvb")
            nc.sync.dma_start(out=bvb_sb[:], in_=bvb.ap())
            bob_sb = cpool.tile([128, MODEL], BF16, tag="bob")
            nc.sync.dma_start(out=bob_sb[:], in_=bob.ap())
            sc2_sb = cpool.tile([128, 2 * S], BF16, tag="sc2")
            nc.sync.dma_start(out=sc2_sb[:], in_=sc2.ap())
            sc_sh_sb = cpool.tile([128, TS], F32, tag="scsh")
            nc.sync.dma_start(out=sc_sh_sb[:], in_=sc_sh.ap())
            tri_sb = cpool.tile([128, 128], BF16, tag="tri")
            nc.sync.dma_start(out=tri_sb[:], in_=tri.ap())
            wup_sb = cpool.tile([128, LJ * 2048], BF16, tag="wup")
            nc.sync.dma_start(out=wup_sb[:], in_=wup.ap())
            wol_sb = cpool.tile([128, HC * MODEL], BF16, tag="wol")
            nc.sync.dma_start(out=wol_sb[:], in_=wol.ap())
            ones_col = cpool.tile([128, 1], BF16, tag="onesc")
            nc.vector.memset(ones_col[:], 1.0)
            ones_row = cpool.tile([1, 128], BF16, tag="onesr")
            nc.vector.memset(ones_row[:], 1.0)

            ag_in = dram.tile([128, AGW], BF16)
            ag_out = dram.tile([G * 128, AGW], BF16)
            rs_in = [dram.tile([S, MODEL // 2], BF16, name=f"rsin{q}")
                     for q in range(2)]
            rs_out = [dram.tile([TS, MODEL // 2], BF16, name=f"rsout{q}")
                      for q in range(2)]

            # ---------- phase A: down-projection on own token shard --------
            with (
                tc.tile_pool(name="phA", bufs=1) as apool,
                tc.tile_pool(name="phAw", bufs=3) as awork,
            ):
                xT_sb = apool.tile([128, 16 * TS], BF16, tag="xT")
                nc.sync.dma_start(out=xT_sb[:], in_=xT.ap())
                lat_sb = apool.tile([128, AGW], BF16, tag="latA")
                nc.vector.memset(lat_sb[PHD:128, 12 * TS:], 0.0)

                for j in range(NLT):
                    wj = awork.tile([128, 2048], BF16, tag="wA", bufs=3,
                                    name=f"wA{j}")
                    nc.sync.dma_start(
                        out=wj[:], in_=w_catp.ap()[:, 2048 * j:2048 * (j + 1)])
                    ps = pspool.tile([128, TS], F32, tag="ps512", bufs=6,
                                     name=f"psA{j}")
                    for m in range(16):
                        nc.tensor.matmul(
                            ps[:], wj[:, 128 * m:128 * (m + 1)],
                            xT_sb[:, TS * m:TS * (m + 1)],
                            start=(m == 0), stop=(m == 15))
                    if j < 12:
                        nc.vector.tensor_scalar_add(
                            lat_sb[:, TS * j:TS * (j + 1)], ps[:],
                            bcon_sb[:, j:j + 1])
                    else:
                        # posk pack: rows 0:64 raw, 64:128 pre-rotated; RoPE.
                        # t3=(raw+b)*cos, t4=(rot+b_rot)*sin_signed (PSUM in0
                        # exempts the equal-base SBUF rule; outputs at base 0)
                        t3 = awork.tile([PHD, TS], F32, tag="pk3", bufs=1,
                                        name="pk3")
                        t4 = awork.tile([PHD, TS], F32, tag="pk4", bufs=1,
                                        name="pk4")
                        nc.vector.scalar_tensor_tensor(
                            t3[:], ps[0:PHD, :], bcon_sb[0:PHD, j:j + 1],
                            sc_sh_sb[0:PHD, :], OP.add, OP.mult)
                        nc.vector.scalar_tensor_tensor(
                            t4[:], ps[PHD:128, :], bcon_sb[PHD:128, j:j + 1],
                            sc_sh_sb[PHD:128, :], OP.add, OP.mult)
                        nc.vector.tensor_tensor(
                            lat_sb[0:PHD, TS * 12:TS * 13], t3[:], t4[:],
                            OP.add)
                nc.sync.dma_start(out=ag_in[:], in_=lat_sb[:])

            nc.gpsimd.collective_compute(
                "AllGather", OP.bypass,
                ins=[ag_in.opt()], outs=[ag_out.opt()],
                replica_groups=groups)

            # ---------- phases B+C+D pool ----------
            with (
                tc.tile_pool(name="phBC", bufs=1) as bpool,
                tc.tile_pool(name="phBCw", bufs=3) as bwork,
            ):
                qT = [bpool.tile([128, S], BF16, tag=f"qT{h}", name=f"qT{h}")
                      for h in range(HC)]
                kT = [bpool.tile([128, S], BF16, tag=f"kT{h}", name=f"kT{h}")
                      for h in range(HC)]
                qpp = [bpool.tile([128, S], BF16, tag=f"qpp{p}",
                                  name=f"qpp{p}") for p in range(2)]
                posk2 = bpool.tile([128, S], BF16, tag="posk2", name="posk2")
                v_sb = [[bpool.tile([128, HD], BF16, tag=f"v{h}_{tt}",
                                    name=f"v{h}_{tt}")
                         for tt in range(S // 128)] for h in range(HC)]
                attnT = [bpool.tile([128, S], BF16, tag=f"at{h}",
                                    name=f"at{h}") for h in range(HC)]

                # ---------- phase B: up-projections ----------
                for r in range(G):
                    cols = slice(TS * r, TS * (r + 1))
                    latr = bwork.tile([128, AGW], BF16, tag="latB", bufs=2,
                                      name=f"latB{r}")
                    nc.sync.dma_start(out=latr[:],
                                      in_=ag_out[128 * r:128 * (r + 1), :])

                    def lq(j):
                        return latr[:, TS * j:TS * (j + 1)]

                    def lk(j):
                        return latr[:, 4 * TS + TS * j:4 * TS + TS * (j + 1)]

                    # q main
                    for h in range(HC):
                        ps = pspool.tile([128, TS], F32, tag="ps512", bufs=6,
                                         name=f"psq{r}{h}")
                        for j in range(LJ):
                            nc.tensor.matmul(
                                ps[:],
                                wup_sb[:, 2048 * j + WQ + HD * h:
                                       2048 * j + WQ + HD * (h + 1)],
                                lq(j)[:], start=(j == 0), stop=(j == LJ - 1))
                        nc.vector.tensor_scalar_add(
                            qT[h][:, cols], ps[:], bcon_sb[:, BQ0 + h:
                                                           BQ0 + h + 1])
                    # q pos (raw + rot per pack), rope combine
                    for p in range(2):
                        psr = pspool.tile([128, TS], F32, tag="ps512", bufs=6,
                                          name=f"pspr{r}{p}")
                        pso = pspool.tile([128, TS], F32, tag="ps512", bufs=6,
                                          name=f"pspo{r}{p}")
                        for j in range(LJ):
                            nc.tensor.matmul(
                                psr[:],
                                wup_sb[:, 2048 * j + WP + 128 * p:
                                       2048 * j + WP + 128 * (p + 1)],
                                lq(j)[:], start=(j == 0), stop=(j == LJ - 1))
                        for j in range(LJ):
                            nc.tensor.matmul(
                                pso[:],
                                wup_sb[:, 2048 * j + WPR + 128 * p:
                                       2048 * j + WPR + 128 * (p + 1)],
                                lq(j)[:], start=(j == 0), stop=(j == LJ - 1))
                        t3 = bwork.tile([128, TS], F32, tag="qp3", bufs=2,
                                        name=f"qp3{r}{p}")
                        t4 = bwork.tile([128, TS], F32, tag="qp4", bufs=2,
                                        name=f"qp4{r}{p}")
                        nc.vector.scalar_tensor_tensor(
                            t3[:], psr[:], bcon_sb[:, BP0 + 2 * p:
                                                   BP0 + 2 * p + 1],
                            sc2_sb[:, cols], OP.add, OP.mult)
                        nc.vector.scalar_tensor_tensor(
                            t4[:], pso[:], bcon_sb[:, BP0 + 2 * p + 1:
                                                   BP0 + 2 * p + 2],
                            sc2_sb[:, S + TS * r:S + TS * (r + 1)],
                            OP.add, OP.mult)
                        nc.vector.tensor_tensor(qpp[p][:, cols], t3[:],
                                                t4[:], OP.add)
                    # k main
                    for h in range(HC):
                        ps = pspool.tile([128, TS], F32, tag="ps512", bufs=6,
                                         name=f"psk{r}{h}")
                        for j in range(LJ):
                            nc.tensor.matmul(
                                ps[:],
                                wup_sb[:, 2048 * j + WK + HD * h:
                                       2048 * j + WK + HD * (h + 1)],
                                lk(j)[:], start=(j == 0), stop=(j == LJ - 1))
                        nc.vector.tensor_scalar_add(
                            kT[h][:, cols], ps[:], bcon_sb[:, BK0 + h:
                                                           BK0 + h + 1])
                    # v token-major (+ bias via broadcast add on eviction)
                    for tt in range(TS // 128):
                        for h in range(HC):
                            psv = pspool.tile([128, HD], F32, tag="ps128",
                                              bufs=2, name=f"psv{r}{tt}{h}")
                            for j in range(LJ):
                                nc.tensor.matmul(
                                    psv[:],
                                    latr[:, 8 * TS + TS * j + 128 * tt:
                                         8 * TS + TS * j + 128 * (tt + 1)],
                                    wup_sb[:, 2048 * j + WV + HD * h:
                                           2048 * j + WV + HD * (h + 1)],
                                    start=(j == 0), stop=(j == LJ - 1))
                            nc.vector.tensor_tensor(
                                v_sb[h][4 * r + tt][:], psv[:],
                                bvb_sb[:, HD * h:HD * (h + 1)], OP.add)
                    # pos_k -> both halves of posk2
                    nc.vector.tensor_copy(posk2[0:PHD, cols],
                                          latr[0:PHD, 12 * TS:13 * TS])
                    nc.vector.tensor_copy(posk2[PHD:128, cols],
                                          latr[0:PHD, 12 * TS:13 * TS])

                # ---------- phase C: attention ----------
                for h in range(HC):
                    p, idx = h // 2, h % 2
                    lo, hi = PHD * idx, PHD * (idx + 1)
                    for u in range(NU):
                        qc0 = TS * u
                        tmax = 4 * u + 3
                        ps_at = pspool.tile([128, TS], F32, tag="ps512",
                                            bufs=6, name=f"psat{h}{u}")
                        ps_sum = pspool.tile([1, TS], F32, tag="ps512",
                                             bufs=6, name=f"pssum{h}{u}")
                        for t in range(tmax + 1):
                            off = 128 * t - TS * u
                            qlo = max(0, off)
                            kc = 128 * t
                            qs = slice(qlo, TS)
                            ps_sc = pspool.tile(
                                [128, TS], F32, tag="ps512", bufs=6,
                                name=f"pssc{h}{u}{t}")
                            nc.tensor.matmul(
                                ps_sc[:, qs], kT[h][:, kc:kc + 128],
                                qT[h][:, qc0 + qlo:qc0 + TS],
                                start=True, stop=False)
                            nc.tensor.matmul(
                                ps_sc[:, qs], posk2[lo:hi, kc:kc + 128],
                                qpp[p][lo:hi, qc0 + qlo:qc0 + TS],
                                start=False, stop=True)
                            pt = bwork.tile([128, TS], BF16, tag="pt",
                                            bufs=4, name=f"pt{h}{u}{t}")
                            nc.scalar.activation(pt[:, qs], ps_sc[:, qs],
                                                 AF.Exp, scale=SCALE)
                            if off >= 0:
                                nc.vector.tensor_tensor(
                                    pt[:, qlo:qlo + 128],
                                    pt[:, qlo:qlo + 128], tri_sb[:],
                                    OP.mult)
                            nc.tensor.matmul(
                                ps_at[:, qs], v_sb[h][t][:], pt[:, qs],
                                start=(t == 0), stop=(t == tmax))
                            nc.tensor.matmul(
                                ps_sum[:, qs], ones_col[:], pt[:, qs],
                                start=(t == 0), stop=(t == tmax))
                        recf = bwork.tile([1, TS], F32, tag="recf",
                                          bufs=2, name=f"recf{h}{u}")
                        nc.vector.reciprocal(recf[:], ps_sum[0:1, :])
                        recb = bwork.tile([1, TS], BF16, tag="recb",
                                          bufs=2, name=f"recb{h}{u}")
                        nc.scalar.copy(recb[:], recf[:])
                        ps_rb = pspool.tile([128, TS], F32, tag="ps512",
                                            bufs=6, name=f"psrb{h}{u}")
                        nc.tensor.matmul(ps_rb[:], ones_row[:], recb[:],
                                         start=True, stop=True)
                        atmp = bwork.tile([128, TS], BF16, tag="atmp",
                                          bufs=2, name=f"atmp{h}{u}")
                        nc.scalar.copy(atmp[:], ps_at[:])
                        nc.vector.tensor_tensor(
                            attnT[h][:, qc0:qc0 + TS], atmp[:], ps_rb[:],
                            OP.mult)

                # ---------- phase D: partial o_proj + ReduceScatter --------
                for q in range(2):
                    for tt in range(S // 128):
                        st = bwork.tile([128, MODEL // 2], BF16, tag="st",
                                        bufs=3, name=f"st{q}{tt}")
                        for oc2 in range(2):
                            oc = 2 * q + oc2
                            ps = pspool.tile([128, TS], F32, tag="ps512",
                                             bufs=6, name=f"psd{oc}{tt}")
                            for h in range(HC):
                                nc.tensor.matmul(
                                    ps[:],
                                    attnT[h][:, 128 * tt:128 * (tt + 1)],
                                    wol_sb[:, MODEL * h + TS * oc:
                                           MODEL * h + TS * (oc + 1)],
                                    start=(h == 0), stop=(h == HC - 1))
                            nc.vector.tensor_tensor(
                                st[:, TS * oc2:TS * (oc2 + 1)],
                                ps[:], bob_sb[:, TS * oc:TS * (oc + 1)],
                                OP.add)
                        nc.sync.dma_start(
                            out=rs_in[q][128 * tt:128 * (tt + 1), :],
                            in_=st[:])
                    nc.gpsimd.collective_compute(
                        "ReduceScatter", OP.add,
                        ins=[rs_in[q].opt()], outs=[rs_out[q].opt()],
                        replica_groups=groups)

                # post-RS: convert to fp32 and write the output shard
                for q in range(2):
                    for tt in range(TS // 128):
                        rt = bwork.tile([128, MODEL // 2], BF16, tag="rt",
                                        bufs=2, name=f"rt{q}{tt}")
                        nc.sync.dma_start(
                            out=rt[:],
                            in_=rs_out[q][128 * tt:128 * (tt + 1), :])
                        ot = bwork.tile([128, MODEL // 2], F32, tag="ot",
                                        bufs=2, name=f"ot{q}{tt}")
                        nc.scalar.copy(ot[:], rt[:])
                        nc.sync.dma_start(
                            out=out_sh.ap()[128 * tt:128 * (tt + 1),
                                            1024 * q:1024 * (q + 1)],
                            in_=ot[:])

    nc.compile()
    return nc


def _host_prep(inputs):
    x = np.asarray(inputs["x"], np.float32)
    w_qkv, b_qkv = inputs["w_qkv"], inputs["b_qkv"]
    w_qup, b_qup = inputs["w_qup"], inputs["b_qup"]
    w_kup, b_kup = inputs["w_kup"], inputs["b_kup"]
    w_vup, b_vup = inputs["w_vup"], inputs["b_vup"]
    w_qpos, b_qpos = inputs["w_qpos"], inputs["b_qpos"]
    w_kpos, b_kpos = inputs["w_kpos"], inputs["b_kpos"]
    w_o, b_o = inputs["w_o"], inputs["b_o"]

    x_flat = x.reshape(T, MODEL)

    # rope tables (position within sequence; same for both batches)
    inv_freq = 1.0 / (THETA ** (np.arange(0, PHD, 2, dtype=np.float32) / PHD))
    pos = np.arange(S, dtype=np.float32)
    freqs = np.outer(pos, inv_freq)
    emb = np.concatenate([freqs, freqs], -1)            # [S, 64]
    cos = np.cos(emb).astype(np.float32)
    sin = np.sin(emb).astype(np.float32)
    sin_signed = np.concatenate([-sin[:, :32], sin[:, 32:]], -1)
    cosT = np.concatenate([cos, cos], 1).T              # [128, S] (2 stacked)
    sinT = np.concatenate([sin_signed, sin_signed], 1).T
    sc2 = np.concatenate([cosT, sinT], 1).astype(BF)    # [128, 2S]

    w_cat = np.concatenate(
        [w_qkv, w_kpos, w_kpos[:, _ROT]], 1).astype(np.float32)  # [2048,1664]
    w_catp = np.ascontiguousarray(
        w_cat.reshape(16, 128, NLT, 128).transpose(1, 2, 0, 3)
        .reshape(128, NLT * 2048)).astype(BF)

    bcat = np.zeros((128, NLT), np.float32)
    for j in range(12):
        bcat[:, j] = b_qkv[128 * j:128 * (j + 1)]
    bcat[0:PHD, 12] = b_kpos
    bcat[PHD:128, 12] = b_kpos[_ROT]

    tri_m = np.triu(np.ones((128, 128), np.float32)).astype(BF)

    bob = np.tile(np.asarray(b_o, np.float32).reshape(1, MODEL) / G,
                  (128, 1)).astype(BF)

    common = {"w_catp": w_catp, "sc2": sc2, "tri": tri_m, "bob": bob}

    in_maps = []
    for c in range(NC):
        w = c % G
        h0 = HC * w
        cm = slice(HD * h0, HD * (h0 + HC))          # 4-head main cols
        cp = slice(PHD * h0, PHD * (h0 + HC))        # 4-head pos cols
        wq = np.asarray(w_qup[:, cm], np.float32)
        wk = np.asarray(w_kup[:, cm], np.float32)
        wv = np.asarray(w_vup[:, cm], np.float32)
        wp = np.asarray(w_qpos[:, cp], np.float32)   # [512, 256]
        wpr = np.concatenate(
            [wp[:, PHD * i:PHD * (i + 1)][:, _ROT] for i in range(HC)], 1)
        wup_l = np.concatenate([
            np.concatenate([wq[128 * j:128 * (j + 1)],
                            wk[128 * j:128 * (j + 1)],
                            wv[128 * j:128 * (j + 1)],
                            wp[128 * j:128 * (j + 1)],
                            wpr[128 * j:128 * (j + 1)]], 1)
            for j in range(LJ)], 1).astype(BF)       # [128, 4*2048]

        # per-core w_o rows (this core's heads), packed [128, h*2048 + c]
        wol_l = np.ascontiguousarray(
            np.asarray(w_o[HD * h0:HD * (h0 + HC), :], np.float32)
            .reshape(HC, 128, MODEL).transpose(1, 0, 2)
            .reshape(128, HC * MODEL)).astype(BF)

        bc = np.zeros((128, BP0 + 4), np.float32)
        bc[:, 0:NLT] = bcat
        for i in range(HC):
            bc[:, BQ0 + i] = b_qup[HD * (h0 + i):HD * (h0 + i + 1)]
            bc[:, BK0 + i] = b_kup[HD * (h0 + i):HD * (h0 + i + 1)]
        for p in range(2):
            bq2 = np.concatenate(
                [b_qpos[PHD * (h0 + 2 * p + i):PHD * (h0 + 2 * p + i + 1)]
                 for i in range(2)])                 # [128]
            bc[:, BP0 + 2 * p] = bq2
            bc[:, BP0 + 2 * p + 1] = np.concatenate(
                [bq2[0:PHD][_ROT], bq2[PHD:128][_ROT]])

        bvb_l = np.tile(np.asarray(b_vup[cm], np.float32).reshape(1, -1),
                        (128, 1)).astype(BF)

        tok = slice(TS * c, TS * (c + 1))
        x_sh = x_flat[tok]                           # [512, 2048]
        xT_l = np.ascontiguousarray(
            x_sh.reshape(TS, 16, 128).transpose(2, 1, 0)
            .reshape(128, 16 * TS)).astype(BF)

        spos = slice(TS * w, TS * (w + 1))           # positions within batch
        scsh = np.concatenate(
            [cosT[0:PHD, spos], sinT[0:PHD, spos]], 0).astype(np.float32)

        m = {"xT": xT_l, "wup": wup_l, "wol": wol_l, "bcon": bc,
             "bvb": bvb_l, "sc_sh": scsh}
        m.update(common)
        in_maps.append(m)
    return in_maps


def kernel(**inputs) -> np.ndarray:
    if "nc" not in _CACHE:
        _CACHE["nc"] = _build()
    nc = _CACHE["nc"]
    in_maps = _host_prep({k: np.asarray(v) for k, v in inputs.items()})
    res = run_bass_kernel_spmd(nc, in_maps, list(range(NC))).results
    out = np.concatenate([res[c]["out_sh"] for c in range(NC)], 0)
    return out.reshape(B, S, MODEL).astype(np.float32)


# revision 21
# speedup vs baseline: 1.4885x; 1.1414x over previous
"""Multi-head latent attention (MLA) Trainium2 kernel, 8-core SPMD.

Sharding: cores split into 2 batch-groups of 4 (cores 0-3 = batch 0,
4-7 = batch 1). Within a group, core w owns token shard [512w, 512w+512)
of its batch and heads {4w..4w+3}.

  - phase A1 (token-parallel): k/v latents + RoPE'd pos_k for the OWN
    token shard; group AllGather (partition-major layout).
  - phase A2 (replicated, overlaps the AllGather): q-latents (lq) for ALL
    batch tokens computed locally - removing lq from the AllGather shrinks
    it by a third, and the redundant compute hides inside the gather.
  - phase B: up-projections. The q-side (qT, RoPE'd positional queries)
    depends only on local lq, so it also overlaps the AllGather; the
    k/v side consumes gathered latents.
    RoPE rotation via pre-permuted weight copies:
    rope(u) = u*cos + perm(u)*sin_signed.
  - phase C: attention in transposed orientation scoresT[k, q]:
    pT = exp(scoresT*scale) feeds attnT = v^T @ pT directly; denominators
    via ones-column matmul; max-free softmax (scores bounded, fp32 exp).
    Span-outer loop order so phase D unblocks span by span.
  - phase D: partial o_proj over local heads for ALL batch tokens
    (+ b_o/4 so the group sum restores the bias once), then per-column
    ReduceScatters hand each core its summed token shard.
All matmul operands bf16, fp32 PSUM accumulation. Host assembles shards.
"""
import numpy as np
import ml_dtypes

import concourse.bacc as bacc
import concourse.mybir as mybir
import concourse.tile as tile
from concourse.bass_utils import run_bass_kernel_spmd

F32 = mybir.dt.float32
BF16 = mybir.dt.bfloat16
AF = mybir.ActivationFunctionType
OP = mybir.AluOpType
BF = ml_dtypes.bfloat16

MODEL = 2048
LATENT = 512
NH = 16
HD = 128          # head dim (main)
PHD = 64          # positional head dim
THETA = 50000.0
B = 2
S = 2048
T = B * S
NC = 8
G = 4             # cores per batch-group
TS = T // NC      # 512 tokens per core shard
HC = NH // G      # 4 heads per core
SCALE = 1.0 / float(np.sqrt(HD + PHD))

LJ = LATENT // 128                # 4 l-chunks per latent
NLT = 3 * LJ + 1                  # 13 w_cat column tiles
AGW = 8 * TS + TS // 2            # 4352: lk(4) + lv(4) + packed posk
NU = S // TS                      # 4 q spans per batch

# bias views into bcon: cols [0:13] b_cat, then q heads, k heads, qpos packs
BQ0, BK0, BP0 = NLT, NLT + HC, NLT + 2 * HC
# wup col layout per j-chunk (stride 2048)
WQ, WK, WV, WP, WPR = 0, 512, 1024, 1536, 1792

_ROT = np.r_[32:64, 0:32]

_CACHE = {}


def _build():
    nc = bacc.Bacc("TRN2", target_bir_lowering=False, debug=False,
                   num_devices=NC)

    xT = nc.dram_tensor("xT", [128, 16 * TS], BF16, kind="ExternalInput")
    xTb = nc.dram_tensor("xTb", [128, 4 * 16 * TS], BF16,
                         kind="ExternalInput")
    w_catp = nc.dram_tensor("w_catp", [128, NLT * 2048], BF16,
                            kind="ExternalInput")
    wup = nc.dram_tensor("wup", [128, LJ * 2048], BF16, kind="ExternalInput")
    wolp = nc.dram_tensor("wolp", [128, HC * MODEL], BF16,
                          kind="ExternalInput")
    bcon = nc.dram_tensor("bcon", [128, BP0 + 4], F32, kind="ExternalInput")
    bvb = nc.dram_tensor("bvb", [128, HC * HD], BF16, kind="ExternalInput")
    bob = nc.dram_tensor("bob", [128, MODEL], BF16, kind="ExternalInput")
    sc2 = nc.dram_tensor("sc2", [128, 2 * S], BF16, kind="ExternalInput")
    sc_sh = nc.dram_tensor("sc_sh", [128, TS], F32, kind="ExternalInput")
    tri = nc.dram_tensor("tri", [128, 128], BF16, kind="ExternalInput")
    out_sh = nc.dram_tensor("out_sh", [TS, MODEL], F32, kind="ExternalOutput")

    groups = [[0, 1, 2, 3], [4, 5, 6, 7]]

    with tile.TileContext(nc) as tc:
        with (
            tc.tile_pool(name="const", bufs=1) as cpool,
            tc.tile_pool(name="psum", bufs=1, space="PSUM") as pspool,
            tc.tile_pool(name="dram", bufs=1, space="DRAM") as dram,
        ):
            # ---------- constants (phase-A-critical first) ----------
            bcon_sb = cpool.tile([128, BP0 + 4], F32, tag="bcon")
            nc.sync.dma_start(out=bcon_sb[:], in_=bcon.ap())
            sc_sh_sb = cpool.tile([128, TS], F32, tag="scsh")
            nc.sync.dma_start(out=sc_sh_sb[:], in_=sc_sh.ap())
            bvb_sb = cpool.tile([128, HC * HD], BF16, tag="bvb")
            bob_sb = cpool.tile([128, MODEL], BF16, tag="bob")
            sc2_sb = cpool.tile([128, 2 * S], BF16, tag="sc2")
            tri_sb = cpool.tile([128, 128], BF16, tag="tri")
            wup_sb = cpool.tile([128, LJ * 2048], BF16, tag="wup")
            ones_col = cpool.tile([128, 1], BF16, tag="onesc")
            nc.vector.memset(ones_col[:], 1.0)
            ones_row = cpool.tile([1, 128], BF16, tag="onesr")
            nc.vector.memset(ones_row[:], 1.0)

            ag_in = dram.tile([128, AGW], BF16)
            ag_out = dram.tile([G * 128, AGW], BF16)
            rs_in = [dram.tile([S, TS], BF16, name=f"rsin{oc}")
                     for oc in range(4)]
            rs_out = [dram.tile([TS, TS], BF16, name=f"rsout{oc}")
                      for oc in range(4)]

            with (
                tc.tile_pool(name="phA", bufs=1) as apool,
                tc.tile_pool(name="phAw", bufs=1) as awork,
            ):
                # ------- phase A1: k/v/posk latents on own token shard ------
                lat_sb = apool.tile([128, AGW], BF16, tag="latA")
                xs = awork.tile([128, 16 * TS], BF16, tag="xs", bufs=2,
                                name="xself")
                nc.sync.dma_start(out=xs[:], in_=xT.ap())
                for jj in range(9):
                    j = 4 + jj          # w_cat tiles 4..12 (lk, lv, posk)
                    wj = awork.tile([128, 2048], BF16, tag="wA", bufs=2,
                                    name=f"wA{j}")
                    nc.sync.dma_start(
                        out=wj[:], in_=w_catp.ap()[:, 2048 * j:2048 * (j + 1)])
                    ps = pspool.tile([128, TS], F32, tag="ps512", bufs=5,
                                     name=f"psA{j}")
                    for m in range(16):
                        nc.tensor.matmul(
                            ps[:], wj[:, 128 * m:128 * (m + 1)],
                            xs[:, TS * m:TS * (m + 1)],
                            start=(m == 0), stop=(m == 15))
                    if j < 12:
                        nc.vector.tensor_scalar_add(
                            lat_sb[:, TS * jj:TS * (jj + 1)], ps[:],
                            bcon_sb[:, j:j + 1])
                    else:
                        # posk pack: rows 0:64 raw, 64:128 pre-rotated; RoPE.
                        # t3=(raw+b)*cos, t4=(rot+b_rot)*sin_signed (PSUM in0
                        # exempts the equal-base SBUF rule)
                        t3 = awork.tile([PHD, TS], F32, tag="pk3", bufs=1,
                                        name="pk3")
                        t4 = awork.tile([PHD, TS], F32, tag="pk4", bufs=1,
                                        name="pk4")
                        nc.vector.scalar_tensor_tensor(
                            t3[:], ps[0:PHD, :], bcon_sb[0:PHD, j:j + 1],
                            sc_sh_sb[0:PHD, :], OP.add, OP.mult)
                        nc.vector.scalar_tensor_tensor(
                            t4[:], ps[PHD:128, :], bcon_sb[PHD:128, j:j + 1],
                            sc_sh_sb[PHD:128, :], OP.add, OP.mult)
                        H = TS // 2
                        nc.vector.tensor_tensor(
                            lat_sb[0:PHD, 8 * TS:8 * TS + H],
                            t3[:, 0:H], t4[:, 0:H], OP.add)
                        nc.vector.tensor_tensor(
                            lat_sb[PHD:128, 8 * TS:8 * TS + H],
                            t3[:, H:TS], t4[:, H:TS], OP.add)
                nc.sync.dma_start(out=ag_in[:], in_=lat_sb[:])

                # deferred constant loads overlap the AllGather
                nc.sync.dma_start(out=wup_sb[:], in_=wup.ap())
                nc.sync.dma_start(out=sc2_sb[:], in_=sc2.ap())
                nc.sync.dma_start(out=bvb_sb[:], in_=bvb.ap())
                nc.sync.dma_start(out=tri_sb[:], in_=tri.ap())
                nc.sync.dma_start(out=bob_sb[:], in_=bob.ap())

                nc.gpsimd.collective_compute(
                    "AllGather", OP.bypass,
                    ins=[ag_in.opt()], outs=[ag_out.opt()],
                    replica_groups=groups)

                # ------- phase A2: lq for ALL batch tokens (overlaps AG) ----
                lqf = [cpool.tile([128, S], BF16, tag=f"lqf{j}",
                                  name=f"lqf{j}") for j in range(LJ)]
                for s in range(4):
                    xb = awork.tile([128, 16 * TS], BF16, tag="xs", bufs=2,
                                    name=f"xb{s}")
                    nc.sync.dma_start(
                        out=xb[:],
                        in_=xTb.ap()[:, 8192 * s:8192 * (s + 1)])
                    for j in range(LJ):
                        wj = awork.tile([128, 2048], BF16, tag="wA", bufs=2,
                                        name=f"wAq{s}{j}")
                        nc.sync.dma_start(
                            out=wj[:],
                            in_=w_catp.ap()[:, 2048 * j:2048 * (j + 1)])
                        ps = pspool.tile([128, TS], F32, tag="ps512", bufs=5,
                                         name=f"psq{s}{j}")
                        for m in range(16):
                            nc.tensor.matmul(
                                ps[:], wj[:, 128 * m:128 * (m + 1)],
                                xb[:, TS * m:TS * (m + 1)],
                                start=(m == 0), stop=(m == 15))
                        nc.vector.tensor_scalar_add(
                            lqf[j][:, TS * s:TS * (s + 1)], ps[:],
                            bcon_sb[:, j:j + 1])

            # ---------- phases B+C+D pool ----------
            with (
                tc.tile_pool(name="phBC", bufs=1) as bpool,
                tc.tile_pool(name="phBCw", bufs=3) as bwork,
            ):
                qT = [bpool.tile([128, S], BF16, tag=f"qT{h}", name=f"qT{h}")
                      for h in range(HC)]
                kT = [bpool.tile([128, S], BF16, tag=f"kT{h}", name=f"kT{h}")
                      for h in range(HC)]
                qpp = [bpool.tile([128, S], BF16, tag=f"qpp{p}",
                                  name=f"qpp{p}") for p in range(2)]
                posk2 = bpool.tile([128, S], BF16, tag="posk2", name="posk2")
                v_sb = [[bpool.tile([128, HD], BF16, tag=f"v{h}_{tt}",
                                    name=f"v{h}_{tt}")
                         for tt in range(S // 128)] for h in range(HC)]
                attnT = [bpool.tile([128, S], BF16, tag=f"at{h}",
                                    name=f"at{h}") for h in range(HC)]

                # ------- phase B q-side (overlaps AG; local lq only) -------
                for s in range(4):
                    cols = slice(TS * s, TS * (s + 1))
                    for h in range(HC):
                        ps = pspool.tile([128, TS], F32, tag="ps512", bufs=5,
                                         name=f"psbq{s}{h}")
                        for j in range(LJ):
                            nc.tensor.matmul(
                                ps[:],
                                wup_sb[:, 2048 * j + WQ + HD * h:
                                       2048 * j + WQ + HD * (h + 1)],
                                lqf[j][:, cols], start=(j == 0),
                                stop=(j == LJ - 1))
                        nc.scalar.activation(
                            qT[h][:, cols], ps[:], AF.Identity,
                            bias=bcon_sb[:, BQ0 + h:BQ0 + h + 1])
                    for p in range(2):
                        psr = pspool.tile([128, TS], F32, tag="ps512", bufs=5,
                                          name=f"pspr{s}{p}")
                        pso = pspool.tile([128, TS], F32, tag="ps512", bufs=5,
                                          name=f"pspo{s}{p}")
                        for j in range(LJ):
                            nc.tensor.matmul(
                                psr[:],
                                wup_sb[:, 2048 * j + WP + 128 * p:
                                       2048 * j + WP + 128 * (p + 1)],
                                lqf[j][:, cols], start=(j == 0),
                                stop=(j == LJ - 1))
                        for j in range(LJ):
                            nc.tensor.matmul(
                                pso[:],
                                wup_sb[:, 2048 * j + WPR + 128 * p:
                                       2048 * j + WPR + 128 * (p + 1)],
                                lqf[j][:, cols], start=(j == 0),
                                stop=(j == LJ - 1))
                        t3 = bwork.tile([128, TS], F32, tag="qpt", bufs=2,
                                        name=f"qp3{s}{p}")
                        t4 = bwork.tile([128, TS], F32, tag="qpt", bufs=2,
                                        name=f"qp4{s}{p}")
                        nc.vector.scalar_tensor_tensor(
                            t3[:], psr[:], bcon_sb[:, BP0 + 2 * p:
                                                   BP0 + 2 * p + 1],
                            sc2_sb[:, cols], OP.add, OP.mult)
                        nc.vector.scalar_tensor_tensor(
                            t4[:], pso[:], bcon_sb[:, BP0 + 2 * p + 1:
                                                   BP0 + 2 * p + 2],
                            sc2_sb[:, S + TS * s:S + TS * (s + 1)],
                            OP.add, OP.mult)
                        nc.vector.tensor_tensor(qpp[p][:, cols], t3[:],
                                                t4[:], OP.add)

                # ------- phase B k/v-side (consumes gathered latents) ------
                for r in range(G):
                    cols = slice(TS * r, TS * (r + 1))
                    latr = bwork.tile([128, AGW], BF16, tag="latB", bufs=2,
                                      name=f"latB{r}")
                    nc.sync.dma_start(out=latr[:],
                                      in_=ag_out[128 * r:128 * (r + 1), :])

                    def lk(j):
                        return latr[:, TS * j:TS * (j + 1)]

                    for h in range(HC):
                        ps = pspool.tile([128, TS], F32, tag="ps512", bufs=5,
                                         name=f"psk{r}{h}")
                        for j in range(LJ):
                            nc.tensor.matmul(
                                ps[:],
                                wup_sb[:, 2048 * j + WK + HD * h:
                                       2048 * j + WK + HD * (h + 1)],
                                lk(j)[:], start=(j == 0), stop=(j == LJ - 1))
                        nc.scalar.activation(
                            kT[h][:, cols], ps[:], AF.Identity,
                            bias=bcon_sb[:, BK0 + h:BK0 + h + 1])
                    # v token-major (+ bias via broadcast add on eviction)
                    for tt in range(TS // 128):
                        for h in range(HC):
                            psv = pspool.tile([128, HD], F32, tag="psx",
                                              bufs=3, name=f"psv{r}{tt}{h}")
                            for j in range(LJ):
                                nc.tensor.matmul(
                                    psv[:],
                                    latr[:, 4 * TS + TS * j + 128 * tt:
                                         4 * TS + TS * j + 128 * (tt + 1)],
                                    wup_sb[:, 2048 * j + WV + HD * h:
                                           2048 * j + WV + HD * (h + 1)],
                                    start=(j == 0), stop=(j == LJ - 1))
                            nc.vector.tensor_tensor(
                                v_sb[h][4 * r + tt][:], psv[:],
                                bvb_sb[:, HD * h:HD * (h + 1)], OP.add)
                    # pos_k -> both halves of posk2 (packed [128, 256])
                    H = TS // 2
                    for half in range(2):
                        nc.vector.tensor_copy(
                            posk2[0:PHD, TS * r + H * half:
                                  TS * r + H * (half + 1)],
                            latr[PHD * half:PHD * (half + 1),
                                 8 * TS:8 * TS + H])
                        nc.vector.tensor_copy(
                            posk2[PHD:128, TS * r + H * half:
                                  TS * r + H * (half + 1)],
                            latr[PHD * half:PHD * (half + 1),
                                 8 * TS:8 * TS + H])

                # ---------- phase C: attention (span-outer) ----------
                for u in range(NU):
                    for h in range(HC):
                        p, idx = h // 2, h % 2
                        lo, hi = PHD * idx, PHD * (idx + 1)
                        qc0 = TS * u
                        tmax = 4 * u + 3
                        ps_at = pspool.tile([128, TS], F32, tag="ps512",
                                            bufs=5, name=f"psat{h}{u}")
                        ps_sum = pspool.tile([1, TS], F32, tag="psx",
                                             bufs=3, name=f"pssum{h}{u}")
                        for t in range(tmax + 1):
                            off = 128 * t - TS * u
                            qlo = max(0, off)
                            kc = 128 * t
                            qs = slice(qlo, TS)
                            ps_sc = pspool.tile(
                                [128, TS], F32, tag="ps512", bufs=5,
                                name=f"pssc{h}{u}{t}")
                            nc.tensor.matmul(
                                ps_sc[:, qs], kT[h][:, kc:kc + 128],
                                qT[h][:, qc0 + qlo:qc0 + TS],
                                start=True, stop=False)
                            nc.tensor.matmul(
                                ps_sc[:, qs], posk2[lo:hi, kc:kc + 128],
                                qpp[p][lo:hi, qc0 + qlo:qc0 + TS],
                                start=False, stop=True)
                            pt = bwork.tile([128, TS], BF16, tag="pt",
                                            bufs=3, name=f"pt{h}{u}{t}")
                            nc.scalar.activation(pt[:, qs], ps_sc[:, qs],
                                                 AF.Exp, scale=SCALE)
                            if off >= 0:
                                nc.vector.tensor_tensor(
                                    pt[:, qlo:qlo + 128],
                                    pt[:, qlo:qlo + 128], tri_sb[:],
                                    OP.mult)
                            nc.tensor.matmul(
                                ps_at[:, qs], v_sb[h][t][:], pt[:, qs],
                                start=(t == 0), stop=(t == tmax))
                            nc.tensor.matmul(
                                ps_sum[:, qs], ones_col[:], pt[:, qs],
                                start=(t == 0), stop=(t == tmax))
                        recf = bwork.tile([1, TS], F32, tag="recf",
                                          bufs=2, name=f"recf{h}{u}")
                        nc.vector.reciprocal(recf[:], ps_sum[0:1, :])
                        recb = bwork.tile([1, TS], BF16, tag="recb",
                                          bufs=2, name=f"recb{h}{u}")
                        nc.scalar.copy(recb[:], recf[:])
                        ps_rb = pspool.tile([128, TS], F32, tag="psx",
                                            bufs=3, name=f"psrb{h}{u}")
                        nc.tensor.matmul(ps_rb[:], ones_row[:], recb[:],
                                         start=True, stop=True)
                        rb_sb = bwork.tile([128, TS], BF16, tag="rbsb",
                                           bufs=2, name=f"rbsb{h}{u}")
                        nc.scalar.copy(rb_sb[:], ps_rb[:])
                        nc.vector.tensor_tensor(
                            attnT[h][:, qc0:qc0 + TS], ps_at[:], rb_sb[:],
                            OP.mult)

                # ---------- phase D: partial o_proj + ReduceScatter --------
                for oc in range(4):
                    wo = bwork.tile([128, MODEL], BF16, tag="wD", bufs=2,
                                    name=f"wD{oc}")
                    nc.sync.dma_start(
                        out=wo[:],
                        in_=wolp.ap()[:, MODEL * oc:MODEL * (oc + 1)])
                    for tt in range(S // 128):
                        ps = pspool.tile([128, TS], F32, tag="ps512",
                                         bufs=5, name=f"psd{oc}{tt}")
                        for h in range(HC):
                            nc.tensor.matmul(
                                ps[:],
                                attnT[h][:, 128 * tt:128 * (tt + 1)],
                                wo[:, TS * h:TS * (h + 1)],
                                start=(h == 0), stop=(h == HC - 1))
                        st = bwork.tile([128, TS], BF16, tag="st", bufs=3,
                                        name=f"st{oc}{tt}")
                        nc.vector.tensor_tensor(
                            st[:], ps[:], bob_sb[:, TS * oc:TS * (oc + 1)],
                            OP.add)
                        nc.sync.dma_start(
                            out=rs_in[oc][128 * tt:128 * (tt + 1), :],
                            in_=st[:])
                    nc.gpsimd.collective_compute(
                        "ReduceScatter", OP.add,
                        ins=[rs_in[oc].opt()], outs=[rs_out[oc].opt()],
                        replica_groups=groups)

                # post-RS: convert to fp32 and write the output shard
                for oc in range(4):
                    for tt in range(TS // 128):
                        rt = bwork.tile([128, TS], BF16, tag="rt",
                                        bufs=2, name=f"rt{oc}{tt}")
                        nc.sync.dma_start(
                            out=rt[:],
                            in_=rs_out[oc][128 * tt:128 * (tt + 1), :])
                        ot = bwork.tile([128, TS], F32, tag="ot",
                                        bufs=2, name=f"ot{oc}{tt}")
                        nc.scalar.copy(ot[:], rt[:])
                        nc.sync.dma_start(
                            out=out_sh.ap()[128 * tt:128 * (tt + 1),
                                            TS * oc:TS * (oc + 1)],
                            in_=ot[:])

    nc.compile()
    return nc


def _host_prep(inputs):
    x = np.asarray(inputs["x"], np.float32)
    w_qkv, b_qkv = inputs["w_qkv"], inputs["b_qkv"]
    w_qup, b_qup = inputs["w_qup"], inputs["b_qup"]
    w_kup, b_kup = inputs["w_kup"], inputs["b_kup"]
    w_vup, b_vup = inputs["w_vup"], inputs["b_vup"]
    w_qpos, b_qpos = inputs["w_qpos"], inputs["b_qpos"]
    w_kpos, b_kpos = inputs["w_kpos"], inputs["b_kpos"]
    w_o, b_o = inputs["w_o"], inputs["b_o"]

    x_flat = x.reshape(T, MODEL)

    # rope tables (position within sequence; same for both batches)
    inv_freq = 1.0 / (THETA ** (np.arange(0, PHD, 2, dtype=np.float32) / PHD))
    pos = np.arange(S, dtype=np.float32)
    freqs = np.outer(pos, inv_freq)
    emb = np.concatenate([freqs, freqs], -1)            # [S, 64]
    cos = np.cos(emb).astype(np.float32)
    sin = np.sin(emb).astype(np.float32)
    sin_signed = np.concatenate([-sin[:, :32], sin[:, 32:]], -1)
    cosT = np.concatenate([cos, cos], 1).T              # [128, S] (2 stacked)
    sinT = np.concatenate([sin_signed, sin_signed], 1).T
    sc2 = np.concatenate([cosT, sinT], 1).astype(BF)    # [128, 2S]

    w_cat = np.concatenate(
        [w_qkv, w_kpos, w_kpos[:, _ROT]], 1).astype(np.float32)  # [2048,1664]
    w_catp = np.ascontiguousarray(
        w_cat.reshape(16, 128, NLT, 128).transpose(1, 2, 0, 3)
        .reshape(128, NLT * 2048)).astype(BF)

    bcat = np.zeros((128, NLT), np.float32)
    for j in range(12):
        bcat[:, j] = b_qkv[128 * j:128 * (j + 1)]
    bcat[0:PHD, 12] = b_kpos
    bcat[PHD:128, 12] = b_kpos[_ROT]

    tri_m = np.triu(np.ones((128, 128), np.float32)).astype(BF)

    bob = np.tile(np.asarray(b_o, np.float32).reshape(1, MODEL) / G,
                  (128, 1)).astype(BF)

    # per-batch xTb: span-major m-major pack of the whole batch
    def pack_xt(x2):                                 # [ntok, MODEL]
        n = x2.shape[0]
        return np.ascontiguousarray(
            x2.reshape(n // TS, TS, 16, 128).transpose(3, 0, 2, 1)
            .reshape(128, (n // TS) * 16 * TS)).astype(BF)

    xTb_g = [pack_xt(x_flat[S * g:S * (g + 1)]) for g in range(B)]

    common = {"w_catp": w_catp, "sc2": sc2, "tri": tri_m, "bob": bob}

    in_maps = []
    for c in range(NC):
        w = c % G
        h0 = HC * w
        cm = slice(HD * h0, HD * (h0 + HC))          # 4-head main cols
        cp = slice(PHD * h0, PHD * (h0 + HC))        # 4-head pos cols
        wq = np.asarray(w_qup[:, cm], np.float32)
        wk = np.asarray(w_kup[:, cm], np.float32)
        wv = np.asarray(w_vup[:, cm], np.float32)
        wp = np.asarray(w_qpos[:, cp], np.float32)   # [512, 256]
        wpr = np.concatenate(
            [wp[:, PHD * i:PHD * (i + 1)][:, _ROT] for i in range(HC)], 1)
        wup_l = np.concatenate([
            np.concatenate([wq[128 * j:128 * (j + 1)],
                            wk[128 * j:128 * (j + 1)],
                            wv[128 * j:128 * (j + 1)],
                            wp[128 * j:128 * (j + 1)],
                            wpr[128 * j:128 * (j + 1)]], 1)
            for j in range(LJ)], 1).astype(BF)       # [128, 4*2048]

        # per-core w_o rows (this core's heads), oc-major:
        # col = 2048*oc + 512*h + c'
        wol_l = np.ascontiguousarray(
            np.asarray(w_o[HD * h0:HD * (h0 + HC), :], np.float32)
            .reshape(HC, 128, 4, TS).transpose(1, 2, 0, 3)
            .reshape(128, HC * MODEL)).astype(BF)

        bc = np.zeros((128, BP0 + 4), np.float32)
        bc[:, 0:NLT] = bcat
        for i in range(HC):
            bc[:, BQ0 + i] = b_qup[HD * (h0 + i):HD * (h0 + i + 1)]
            bc[:, BK0 + i] = b_kup[HD * (h0 + i):HD * (h0 + i + 1)]
        for p in range(2):
            bq2 = np.concatenate(
                [b_qpos[PHD * (h0 + 2 * p + i):PHD * (h0 + 2 * p + i + 1)]
                 for i in range(2)])                 # [128]
            bc[:, BP0 + 2 * p] = bq2
            bc[:, BP0 + 2 * p + 1] = np.concatenate(
                [bq2[0:PHD][_ROT], bq2[PHD:128][_ROT]])

        bvb_l = np.tile(np.asarray(b_vup[cm], np.float32).reshape(1, -1),
                        (128, 1)).astype(BF)

        tok = slice(TS * c, TS * (c + 1))
        xT_l = pack_xt(x_flat[tok])                  # [128, 16*TS]

        spos = slice(TS * w, TS * (w + 1))           # positions within batch
        scsh = np.concatenate(
            [cosT[0:PHD, spos], sinT[0:PHD, spos]], 0).astype(np.float32)

        m = {"xT": xT_l, "xTb": xTb_g[c // G], "wup": wup_l, "wolp": wol_l,
             "bcon": bc, "bvb": bvb_l, "sc_sh": scsh}
        m.update(common)
        in_maps.append(m)
    return in_maps


def kernel(**inputs) -> np.ndarray:
    if "nc" not in _CACHE:
        _CACHE["nc"] = _build()
    nc = _CACHE["nc"]
    in_maps = _host_prep({k: np.asarray(v) for k, v in inputs.items()})
    res = run_bass_kernel_spmd(nc, in_maps, list(range(NC))).results
    out = np.concatenate([res[c]["out_sh"] for c in range(NC)], 0)
    return out.reshape(B, S, MODEL).astype(np.float32)
